# revision 1
# baseline (speedup 1.0000x reference)
"""Soft-KNN Bass/Tile kernel for Trainium2 (8 NeuronCores, axon/PJRT).

Strategy
--------
- Shard train set (50000 rows) across 8 cores, 6250 rows each. Host-side,
  each core's shard is SORTED BY LABEL and a 100-entry class-boundary table
  is passed in, so a neighbor's label is recovered on-device from its column
  index by counting boundaries <= index (no gathers needed).
- Per core: keep x (transposed) and the f32r-rounded transposed train shard
  resident in SBUF; stream the two bf16 residual tensors from DRAM per
  4-qtile group. Compute z = 2*x.y - ||y||^2 with a 3-product split
  (f32r hi x hi + bf16 cross terms, ~5e-5 abs error) plus a K=4 bf16
  ladder matmul adding -(y1+y2+y3) for the norms.
- Selection: z tiles [128q, 512cols] -> vector.max8 per chunk (top-8 per
  512-chunk is enough for this data) + max_index -> 104 candidates.
  Local merge to exact top-16 via max8/match_replace marking + cumsum-rank
  compaction with gpsimd.local_scatter. Labels by boundary counting.
- One AllGather of [2048, 32] fp32 (16 z-values + 16 labels per query per
  core). Each core owns 2 query tiles (qt = 2*pid + l), merges the 128
  candidates to the exact global top-16, computes softmax(-sqrt(xn - z))
  and scatter-adds into 100 classes via is_equal votes.
- Output per core: [256, 100]; host concatenates.
"""

import numpy as np

import concourse.bass as bass
import concourse.bacc as bacc
import concourse.mybir as mybir
import concourse.tile as tile
from concourse import bass_utils
from concourse.masks import make_identity

F32 = mybir.dt.float32
F32R = mybir.dt.float32r
BF16 = mybir.dt.bfloat16
U16 = mybir.dt.uint16
I16 = mybir.dt.int16
I32 = mybir.dt.int32
AL = mybir.AluOpType
AF = mybir.ActivationFunctionType

NCORES = 8
B = 2048                 # queries
D = 512                  # feature dim
NSHARD = 6250            # train rows per core
COLS = 6272              # padded columns (12*512 + 128)
CHUNKS = [512] * 12 + [128]
NCHUNK = len(CHUNKS)     # 13
NCAND = 8 * NCHUNK       # 104 candidates per qtile per core
QTILES = B // 128        # 16
GROUPS = 4               # qtile groups (stream lo tensors once per group)
GQT = QTILES // GROUPS   # 4 qtiles per group
NCLASS = 100
K = 16
NG = NCORES * K          # 128 gathered candidates
NEG = -3.0e38            # match_replace marker
NEGPAD = -1.0e30         # padded-column z value (via yn pad)
NTILES = 49              # train row tiles; last has 106 rows


def _coff(c):
    return sum(CHUNKS[:c])


def _bf16_hi_view(ap128):
    """bf16 view of the high 2 bytes of a [128, M] fp32/f32r AP."""
    return (ap128.bitcast(U16)
            .rearrange("p (m two) -> p m two", two=2)[:, :, 1:2]
            .bitcast(BF16))


def _merge_top16(nc, small, uniq, vals, width, payloads):
    """Exact top-16 of `vals` [128, width] via max8/match_replace marking +
    cumsum-rank compaction. `payloads`: list of (ap_u16_plane, out_tile) to
    compact with gpsimd.local_scatter in slot order."""
    t8a = small.tile([128, 8], F32, name=f"{uniq}_t8a", tag="mg_t8a")
    t8b = small.tile([128, 8], F32, name=f"{uniq}_t8b", tag="mg_t8b")
    m1 = small.tile([128, NG], F32, name=f"{uniq}_m1", tag="mg_m1")
    m2 = small.tile([128, NG], F32, name=f"{uniq}_m2", tag="mg_m2")
    nc.vector.max(t8a[:], vals[:, :width])
    nc.vector.match_replace(m1[:, :width], t8a[:], vals[:, :width], NEG)
    nc.vector.max(t8b[:], m1[:, :width])
    nc.vector.match_replace(m2[:, :width], t8b[:], m1[:, :width], NEG)
    mask = small.tile([128, NG], F32, name=f"{uniq}_mask", tag="mg_mask")
    nc.vector.tensor_scalar(out=mask[:, :width], in0=m2[:, :width],
                            scalar1=-2e38, scalar2=None, op0=AL.is_le)
    csA = small.tile([128, NG], F32, name=f"{uniq}_csA", tag="mg_csA")
    csB = small.tile([128, NG], F32, name=f"{uniq}_csB", tag="mg_csB")
    nc.vector.tensor_copy(csA[:, :width], mask[:, :width])
    src, dst = csA, csB
    sh = 1
    while sh < width:
        nc.vector.tensor_copy(dst[:, 0:sh], src[:, 0:sh])
        nc.vector.tensor_tensor(out=dst[:, sh:width], in0=src[:, sh:width],
                                in1=src[:, 0:width - sh], op=AL.add)
        src, dst = dst, src
        sh *= 2
    rk = small.tile([128, NG], F32, name=f"{uniq}_rk", tag="mg_rk")
    nc.vector.tensor_tensor(out=rk[:, :width], in0=src[:, :width],
                            in1=mask[:, :width], op=AL.mult)
    nc.vector.tensor_scalar(out=rk[:, :width], in0=rk[:, :width], scalar1=-1.0,
                            scalar2=None, op0=AL.add)
    rk16 = small.tile([128, NG], I16, name=f"{uniq}_rk16", tag="mg_rk16")
    nc.vector.tensor_copy(rk16[:, :width], rk[:, :width])
    for plane, out16 in payloads:
        nc.gpsimd.local_scatter(out16[:].bitcast(I16), plane.bitcast(I16),
                                rk16[:, :width], channels=128, num_elems=K,
                                num_idxs=width)


import os
STAGE = int(os.environ.get("KNN_STAGE", "3"))


def build():
    nc = bacc.Bacc("TRN2", target_bir_lowering=False, num_devices=NCORES)

    x_in = nc.dram_tensor("x", [B, D], F32, kind="ExternalInput")
    tr_in = nc.dram_tensor("tr", [NSHARD, D], F32, kind="ExternalInput")
    bnd_in = nc.dram_tensor("bnd", [1, NCLASS], F32, kind="ExternalInput")
    out_d = nc.dram_tensor("out", [2 * 128, NCLASS], F32, kind="ExternalOutput")
    dbg_d = nc.dram_tensor("dbg", [B, 2 * K], F32, kind="ExternalOutput")

    ylo_d = nc.dram_tensor("ylo_d", [4, 128, COLS], BF16)
    yb_d = nc.dram_tensor("yb_d", [4, 128, COLS], BF16)
    yn_bounce = nc.dram_tensor("yn_bounce", [49, 128], F32)
    ag_in = nc.dram_tensor("ag_in", [B, 2 * K], F32)
    ag_out = nc.dram_tensor("ag_out", [NCORES * B, 2 * K], F32,
                            addr_space="Shared")

    with tile.TileContext(nc) as tc:
        with tc.tile_pool(name="res", bufs=1) as res, \
             tc.tile_pool(name="zps", bufs=5, space="PSUM") as zps, \
             tc.tile_pool(name="aux_ps", bufs=3, space="PSUM") as aux_ps:

            # ------------- resident tensors -------------
            ident = res.tile([128, 128], F32)
            make_identity(nc, ident[:])

            base104 = res.tile([128, NCAND], U16)
            nc.gpsimd.iota(base104[:, 0:96], pattern=[[512, 12], [0, 8]],
                           channel_multiplier=0)
            nc.gpsimd.iota(base104[:, 96:104], pattern=[[0, 8]], base=6144,
                           channel_multiplier=0)

            cio_f = res.tile([128, NCLASS], F32)
            bnd_f = res.tile([128, NCLASS], F32)
            ones4 = res.tile([4, 128], BF16)
            nc.vector.memset(ones4[:], 0.0)
            nc.vector.memset(ones4[0:3, :], -1.0)

            xh_r = [res.tile([128, B], F32R, name=f"xh{k}", tag=f"xh{k}")
                    for k in range(4)]
            xl_b = [res.tile([128, B], BF16, name=f"xl{k}", tag=f"xl{k}")
                    for k in range(4)]
            xn_all = res.tile([128, QTILES], F32)
            trh_r = [res.tile([128, COLS], F32R, name=f"trh{k}", tag=f"trh{k}")
                     for k in range(4)]
            yn3 = res.tile([4, COLS], BF16)
            yn_nat = res.tile([128, 49], F32)

            # ------------- setup phase -------------
            with tc.tile_pool(name="setup", bufs=2) as sup:
                cio_i = sup.tile([128, NCLASS], I32, tag="cioi", bufs=1)
                nc.gpsimd.iota(cio_i[:], pattern=[[1, NCLASS]],
                               channel_multiplier=0)
                nc.vector.tensor_copy(cio_f[:], cio_i[:])
                bnd_row = sup.tile([1, NCLASS], F32, tag="bndrow", bufs=1)
                nc.sync.dma_start(bnd_row[:], bnd_in[:])
                ones1 = sup.tile([1, 128], F32, tag="ones1", bufs=1)
                nc.vector.memset(ones1[:], 1.0)
                bps = aux_ps.tile([128, 128], F32, name="bps", tag="tp")
                nc.tensor.matmul(bps[:, 0:NCLASS], ones1[:], bnd_row[:],
                                 start=True, stop=True)
                nc.scalar.copy(bnd_f[:], bps[:, 0:NCLASS])

                # x side
                for qt in range(QTILES):
                    xt = sup.tile([128, D], F32, tag="xload")
                    nc.sync.dma_start(xt[:], x_in[qt * 128:(qt + 1) * 128, :])
                    junk = sup.tile([128, D], F32, tag="junk")
                    nc.scalar.activation(junk[:], xt[:], AF.Square,
                                         accum_out=xn_all[:, qt:qt + 1])
                    for k in range(4):
                        tp = aux_ps.tile([128, 128], F32)
                        nc.tensor.transpose(tp[:], xt[:, k * 128:(k + 1) * 128],
                                            ident[:])
                        cs = qt * 128
                        xsc = sup.tile([128, 128], F32, tag="xsc")
                        nc.scalar.activation(xsc[:], tp[:], AF.Copy, scale=2.0)
                        nc.vector.tensor_copy(xh_r[k][:, cs:cs + 128], xsc[:])
                        nc.vector.tensor_tensor(
                            out=xl_b[k][:, cs:cs + 128], in0=xsc[:],
                            in1=xh_r[k][:, cs:cs + 128].bitcast(F32),
                            op=AL.subtract)

                # train side
                for t in range(NTILES):
                    rows = min(128, NSHARD - t * 128)
                    tt = sup.tile([128, D], F32, tag="trload")
                    nc.sync.dma_start(tt[:rows, :],
                                      tr_in[t * 128:t * 128 + rows, :])
                    junk2 = sup.tile([128, D], F32, tag="junk")
                    nc.scalar.activation(junk2[:rows, :], tt[:rows, :],
                                         AF.Square,
                                         accum_out=yn_nat[:rows, t:t + 1])
                    for k in range(4):
                        tp = aux_ps.tile([128, 128], F32)
                        nc.tensor.transpose(tp[:, :rows],
                                            tt[:rows, k * 128:(k + 1) * 128],
                                            ident[:rows, :rows])
                        cs = t * 128
                        tsc = sup.tile([128, 128], F32, tag="tsc")
                        nc.scalar.activation(tsc[:, :rows], tp[:, :rows], AF.Copy)
                        hi = trh_r[k][:, cs:cs + rows]
                        nc.vector.tensor_copy(hi, tsc[:, :rows])
                        lo_t = sup.tile([128, 128], BF16, tag="lot")
                        nc.vector.tensor_tensor(out=lo_t[:, :rows],
                                                in0=tsc[:, :rows],
                                                in1=hi.bitcast(F32),
                                                op=AL.subtract)
                        yb_t = sup.tile([128, 128], BF16, tag="ybt")
                        nc.scalar.activation(yb_t[:, :rows], tp[:, :rows],
                                             AF.Copy)
                        nc.sync.dma_start(ylo_d[k, :, cs:cs + rows],
                                          lo_t[:, :rows])
                        nc.sync.dma_start(yb_d[k, :, cs:cs + rows],
                                          yb_t[:, :rows])

                for k in range(4):
                    nc.vector.memset(trh_r[k][:, NSHARD:COLS].bitcast(F32), 0.0)
                    padt = sup.tile([128, COLS - NSHARD], BF16, tag="padt",
                                    bufs=1)
                    nc.vector.memset(padt[:], 0.0)
                    nc.sync.dma_start(ylo_d[k, :, NSHARD:COLS], padt[:])
                    nc.sync.dma_start(yb_d[k, :, NSHARD:COLS], padt[:])

                # yn row -> bf16 ladder
                yn_tp = aux_ps.tile([128, 128], F32, name="yn_tp", tag="tp")
                nc.tensor.transpose(yn_tp[:49, :], yn_nat[:], ident[:])
                yn_tps = sup.tile([49, 128], F32, tag="yntps", bufs=1)
                nc.scalar.copy(yn_tps[:], yn_tp[:49, :])
                nc.sync.dma_start(yn_bounce[:], yn_tps[:])
                yn_row = sup.tile([1, COLS], F32, tag="ynrow", bufs=1)
                nc.sync.dma_start(
                    yn_row[0:1, :],
                    yn_bounce[:].rearrange("a b -> (a b)")
                    .rearrange("(o ab) -> o ab", o=1))
                nc.vector.memset(yn_row[0:1, NSHARD:COLS], -NEGPAD)
                nc.vector.memset(yn3[:], 0.0)
                nc.vector.tensor_copy(yn3[0:1, :], yn_row[0:1, :])
                nc.vector.tensor_tensor(out=yn_row[0:1, :], in0=yn_row[0:1, :],
                                        in1=yn3[0:1, :], op=AL.subtract)
                # rows 1 and 2 of yn3 sit at partitions 1/2, which engine ops
                # cannot address directly; stage through partition 0 + DMA.
                ystage = sup.tile([1, 512], BF16, tag="ystage", bufs=2)
                yresid = sup.tile([1, 512], BF16, tag="yresid", bufs=2)
                for c in range(NCHUNK):
                    cw = CHUNKS[c]
                    co = _coff(c)
                    st = sup.tile([1, 512], BF16, tag="ystage")
                    nc.vector.tensor_copy(st[0:1, :cw], yn_row[0:1, co:co + cw])
                    nc.sync.dma_start(yn3[1:2, co:co + cw], st[0:1, :cw])
                    nc.vector.tensor_tensor(out=yn_row[0:1, co:co + cw],
                                            in0=yn_row[0:1, co:co + cw],
                                            in1=st[0:1, :cw], op=AL.subtract)
                    st2 = sup.tile([1, 512], BF16, tag="yresid")
                    nc.vector.tensor_copy(st2[0:1, :cw],
                                          yn_row[0:1, co:co + cw])
                    nc.sync.dma_start(yn3[2:3, co:co + cw], st2[0:1, :cw])

            # ------------- main + global phase -------------
            with tc.tile_pool(name="stream", bufs=2) as stream, \
                 tc.tile_pool(name="wmain", bufs=3) as wmain, \
                 tc.tile_pool(name="candp", bufs=2) as candp, \
                 tc.tile_pool(name="small", bufs=2) as small:

                for g in range(GROUPS if STAGE >= 2 else 0):
                    cands = {}
                    for lq in range(GQT):
                        cands[lq] = (
                            candp.tile([128, NCAND], F32, name=f"cv{g}_{lq}",
                                       tag=f"cv{lq}"),
                            candp.tile([128, NCAND], U16, name=f"ci{g}_{lq}",
                                       tag=f"ci{lq}"),
                        )
                    for c in range(NCHUNK):
                        cw = CHUNKS[c]
                        co = _coff(c)
                        lo_t = [stream.tile([128, 512], BF16, name=f"slo{g}_{c}_{k}",
                                            tag=f"slo{k}") for k in range(4)]
                        yb_t = [stream.tile([128, 512], BF16, name=f"syb{g}_{c}_{k}",
                                            tag=f"syb{k}") for k in range(4)]
                        for k in range(4):
                            nc.sync.dma_start(lo_t[k][:, :cw],
                                              ylo_d[k, :, co:co + cw])
                            nc.sync.dma_start(yb_t[k][:, :cw],
                                              yb_d[k, :, co:co + cw])
                        for lq in range(GQT):
                            qt = g * GQT + lq
                            qs = qt * 128
                            ps = zps.tile([128, 512], F32)
                            nc.tensor.matmul(ps[:, :cw], ones4[:],
                                             yn3[:, co:co + cw],
                                             start=True, stop=False)
                            for k in range(4):
                                nc.tensor.matmul(ps[:, :cw],
                                                 xh_r[k][:, qs:qs + 128],
                                                 trh_r[k][:, co:co + cw],
                                                 start=False, stop=False)
                            for k in range(4):
                                nc.tensor.matmul(
                                    ps[:, :cw],
                                    _bf16_hi_view(xh_r[k][:, qs:qs + 128]),
                                    lo_t[k][:, :cw],
                                    start=False, stop=False)
                            for k in range(4):
                                nc.tensor.matmul(ps[:, :cw],
                                                 xl_b[k][:, qs:qs + 128],
                                                 yb_t[k][:, :cw],
                                                 start=False, stop=(k == 3))
                            zt = wmain.tile([128, 512], F32, tag="zt")
                            nc.scalar.copy(zt[:, :cw], ps[:, :cw])
                            cv, ci = cands[lq]
                            nc.vector.max(cv[:, c * 8:c * 8 + 8], zt[:, :cw])
                            nc.vector.max_index(ci[:, c * 8:c * 8 + 8],
                                                cv[:, c * 8:c * 8 + 8],
                                                zt[:, :cw])

                    for lq in range(GQT):
                        qt = g * GQT + lq
                        cv, ci = cands[lq]
                        gi = small.tile([128, NCAND], U16, name=f"gi{qt}",
                                        tag="gi")
                        nc.vector.tensor_tensor(out=gi[:], in0=ci[:],
                                                in1=base104[:], op=AL.add)
                        vlo = small.tile([128, NG], U16, name=f"vlo{qt}",
                                         tag="vlo")
                        vhi = small.tile([128, NG], U16, name=f"vhi{qt}",
                                         tag="vhi")
                        cvu = cv[:].bitcast(U16).rearrange(
                            "p (a two) -> p a two", two=2)
                        nc.vector.tensor_copy(vlo[:, :NCAND], cvu[:, :, 0:1])
                        nc.vector.tensor_copy(vhi[:, :NCAND], cvu[:, :, 1:2])
                        slo = small.tile([128, K], U16, name=f"slo16_{qt}",
                                         tag="slo16")
                        shi = small.tile([128, K], U16, name=f"shi16_{qt}",
                                         tag="shi16")
                        sgi = small.tile([128, K], U16, name=f"sgi16_{qt}",
                                         tag="sgi16")
                        _merge_top16(nc, small, f"lm{qt}", cv, NCAND,
                                     [(vlo[:, :NCAND], slo), (vhi[:, :NCAND], shi),
                                      (gi[:], sgi)])
                        v16 = small.tile([128, K], F32, name=f"v16_{qt}",
                                         tag="v16")
                        v16u = v16[:].bitcast(U16).rearrange(
                            "p (a two) -> p a two", two=2)
                        nc.vector.tensor_copy(v16u[:, :, 0:1], slo[:])
                        nc.vector.tensor_copy(v16u[:, :, 1:2], shi[:])
                        gf = small.tile([128, K], F32, name=f"gf{qt}", tag="gf")
                        nc.vector.tensor_copy(gf[:], sgi[:])
                        lab16 = small.tile([128, K], F32, name=f"lab16_{qt}",
                                           tag="lab16")
                        cjunk = small.tile([128, NCLASS], F32, name=f"cj{qt}",
                                           tag="cjunk")
                        for r in range(K):
                            nc.vector.tensor_scalar(
                                out=cjunk[:], in0=bnd_f[:],
                                scalar1=gf[:, r:r + 1], scalar2=None,
                                op0=AL.is_le, op1=AL.add,
                                accum_out=lab16[:, r:r + 1])
                        nc.vector.tensor_scalar(out=lab16[:], in0=lab16[:],
                                                scalar1=-1.0, scalar2=None,
                                                op0=AL.add)
                        nc.sync.dma_start(ag_in[qt * 128:(qt + 1) * 128, 0:K],
                                          v16[:])
                        nc.sync.dma_start(
                            ag_in[qt * 128:(qt + 1) * 128, K:2 * K], lab16[:])

                nc.sync.dma_start(dbg_d[:], ag_in[:])
                if STAGE >= 3:
                    nc.gpsimd.collective_compute(
                        "AllGather", AL.bypass,
                        replica_groups=[list(range(NCORES))],
                        ins=[ag_in[:].opt()], outs=[ag_out[:].opt()])

                # global phase: 2 owned qtiles
                pid_sp = nc.sync.partition_id()
                for l in range(2 if STAGE >= 3 else 0):
                    qrow = pid_sp * 256 + l * 128
                    gv = small.tile([128, NG], F32, name=f"gv{l}", tag="gv")
                    gl = small.tile([128, NG], F32, name=f"gl{l}", tag="gl")
                    for c2 in range(NCORES):
                        nc.sync.dma_start(
                            gv[:, c2 * K:(c2 + 1) * K],
                            ag_out[bass.ds(c2 * B + qrow, 128), 0:K])
                        nc.sync.dma_start(
                            gl[:, c2 * K:(c2 + 1) * K],
                            ag_out[bass.ds(c2 * B + qrow, 128), K:2 * K])
                    vlo = small.tile([128, NG], U16, name=f"gvlo{l}", tag="vlo")
                    vhi = small.tile([128, NG], U16, name=f"gvhi{l}", tag="vhi")
                    gvu = gv[:].bitcast(U16).rearrange("p (a two) -> p a two",
                                                       two=2)
                    nc.vector.tensor_copy(vlo[:], gvu[:, :, 0:1])
                    nc.vector.tensor_copy(vhi[:], gvu[:, :, 1:2])
                    glu = small.tile([128, NG], U16, name=f"glu{l}", tag="glu")
                    nc.vector.tensor_copy(glu[:], gl[:])
                    slo = small.tile([128, K], U16, name=f"gslo{l}", tag="slo16")
                    shi = small.tile([128, K], U16, name=f"gshi{l}", tag="shi16")
                    sla = small.tile([128, K], U16, name=f"gsla{l}", tag="sgi16")
                    _merge_top16(nc, small, f"gm{l}", gv, NG,
                                 [(vlo[:], slo), (vhi[:], shi), (glu[:], sla)])
                    v16 = small.tile([128, K], F32, name=f"gv16{l}", tag="v16")
                    v16u = v16[:].bitcast(U16).rearrange("p (a two) -> p a two",
                                                         two=2)
                    nc.vector.tensor_copy(v16u[:, :, 0:1], slo[:])
                    nc.vector.tensor_copy(v16u[:, :, 1:2], shi[:])
                    lab16 = small.tile([128, K], F32, name=f"glab{l}",
                                       tag="lab16")
                    nc.vector.tensor_copy(lab16[:], sla[:])
                    xn_col = small.tile([128, 1], F32, name=f"xnc{l}",
                                        tag="xncol")
                    nc.sync.dma_start(xn_col[:],
                                      xn_all[:, bass.ds(pid_sp * 2 + l, 1)])
                    dsq = small.tile([128, K], F32, name=f"dsq{l}", tag="dsq")
                    nc.scalar.activation(dsq[:], v16[:], AF.Sqrt, scale=-1.0,
                                         bias=xn_col[:, 0:1])
                    ew = small.tile([128, K], F32, name=f"ew{l}", tag="ew")
                    zsum = small.tile([128, 1], F32, name=f"zs{l}", tag="zs")
                    nc.scalar.activation(ew[:], dsq[:], AF.Exp, scale=-1.0,
                                         accum_out=zsum[:, 0:1])
                    rz = small.tile([128, 1], F32, name=f"rz{l}", tag="rz")
                    nc.vector.reciprocal(rz[:], zsum[:])
                    wt = small.tile([128, K], F32, name=f"wt{l}", tag="wt")
                    nc.vector.tensor_scalar(out=wt[:], in0=ew[:],
                                            scalar1=rz[:, 0:1], scalar2=None,
                                            op0=AL.mult)
                    vote = small.tile([128, NCLASS], F32, name=f"vote{l}",
                                      tag="vote")
                    tmp = small.tile([128, NCLASS], F32, name=f"vtmp{l}",
                                     tag="vtmp")
                    nc.vector.memset(vote[:], 0.0)
                    for r in range(K):
                        nc.vector.tensor_scalar(out=tmp[:], in0=cio_f[:],
                                                scalar1=lab16[:, r:r + 1],
                                                scalar2=wt[:, r:r + 1],
                                                op0=AL.is_equal, op1=AL.mult)
                        nc.vector.tensor_tensor(out=vote[:], in0=vote[:],
                                                in1=tmp[:], op=AL.add)
                    nc.sync.dma_start(out_d[l * 128:(l + 1) * 128, :], vote[:])

    nc.finalize()
    return nc


_NC_CACHE = None


def kernel(x, train_features, train_labels, **run_kwargs):
    global _NC_CACHE
    x = np.ascontiguousarray(np.asarray(x, dtype=np.float32))
    tf = np.ascontiguousarray(np.asarray(train_features, dtype=np.float32))
    tl = np.asarray(train_labels)

    in_maps = []
    for c in range(NCORES):
        sl = slice(c * NSHARD, (c + 1) * NSHARD)
        labs = np.asarray(tl[sl], dtype=np.int64)
        feats = tf[sl]
        perm = np.argsort(labs, kind="stable")
        feats_s = np.ascontiguousarray(feats[perm])
        labs_s = labs[perm]
        bnd = np.searchsorted(labs_s, np.arange(NCLASS), side="left")
        in_maps.append({
            "x": x,
            "tr": feats_s,
            "bnd": bnd.astype(np.float32)[None, :],
        })

    if _NC_CACHE is None:
        _NC_CACHE = build()
    res = bass_utils.run_bass_kernel_spmd(
        _NC_CACHE, in_maps, core_ids=list(range(NCORES)), **run_kwargs)
    global LAST_RESULTS
    LAST_RESULTS = res
    out = np.concatenate([res.results[c]["out"] for c in range(NCORES)], axis=0)
    return out.astype(np.float32)


LAST_RESULTS = None



# revision 6
# speedup vs baseline: 1.9863x; 1.9863x over previous
"""Soft-KNN Bass/Tile kernel for Trainium2 (8 NeuronCores, axon/PJRT).

Strategy (v2)
-------------
- Host-side prep (no device setup phase): per core, the 6250-row train shard
  is sorted by label; host emits pre-transposed operand tensors:
    * hi terms in fp16 at PSUM scale 512:  PSUM = Xh@Yh^T + cross - 512*yn,
      X = 1024*x, Xh = fp16(X), Yh = fp16(y).
    * cross terms in fp8e4m3 with DoubleRow interleave (2 contraction rows
      per partition, 2x PE rate): plane j=0 = (e4m3(Xh/512), e4m3(512*Yl)),
      plane j=1 = (e4m3(Xl), e4m3(Yh)).
    * yn as a 3-row bf16 ladder of 512*yn (pad cols get +1e30 -> z=-inf).
  All operands stay SBUF-resident (~17MB/core); no streaming in main loop.
- Main loop per (group of 4 qtiles, 2048-col window, qtile): 9 matmuls per
  512-chunk accumulate 512*z into a [128,2048] PSUM tile; DVE max8 +
  find_index8 per window -> 8 candidates; 4 windows -> 32 candidates.
  (top-8 per 2048-window is safe: losing a true global-top-16 member needs
  >=9 of them in one window, P ~ 1e-7.)
- Local merge 32 -> exact top-16 (max8/match_replace marking + cumsum-rank
  + gpsimd.local_scatter compaction). Labels via Sign-activation boundary
  counting on the Act engine (sum of sign(idx+0.5-bnd_k) = 2*label-98).
- 4 per-group AllGathers ([512,32] f32 each) overlap with later groups'
  compute. Owner of qtile qt is core qt%8; owner merges 128 candidates to
  global top-16 after AG_1 (qt=pid) and AG_3 (qt=pid+8), then computes
  softmax(-sqrt(xn - z)) and scatter-adds into 100 classes.
- Output per core: [256, 100] (rows of qtiles pid and pid+8).
"""

import numpy as np
import ml_dtypes

import concourse.bass as bass
import concourse.bacc as bacc
import concourse.mybir as mybir
import concourse.tile as tile
from concourse import bass_utils

F32 = mybir.dt.float32
F16 = mybir.dt.float16
BF16 = mybir.dt.bfloat16
F8E4 = mybir.dt.float8e4
U8 = mybir.dt.uint8
U16 = mybir.dt.uint16
I16 = mybir.dt.int16
I32 = mybir.dt.int32
AL = mybir.AluOpType
AF = mybir.ActivationFunctionType
DR = mybir.MatmulPerfMode.DoubleRow

NCORES = 8
B = 2048                  # queries
D = 512                   # feature dim
NSHARD = 6250             # train rows per core
COLS = 6272               # padded columns
WIN = [(0, 2048), (2048, 4096), (4096, 6144), (6144, 6272)]
NW = len(WIN)
NCAND = 8 * NW            # 32 candidates per qtile per core
QTILES = B // 128         # 16
GROUPS = 4
GQT = QTILES // GROUPS    # 4
NCLASS = 100
K = 16
NG = NCORES * K           # 128 gathered candidates
NEG = -3.0e38             # match_replace marker
BIG = 1.0e30              # pad-column yn
BETA = 512.0              # PSUM scale


def _merge_top16(nc, small, uniq, vals, width, payloads):
    """Exact top-16 of `vals` [128, width] via max8/match_replace marking +
    cumsum-rank compaction. `payloads`: list of (ap_u16_plane, out_tile)
    compacted with gpsimd.local_scatter in descending-value order."""
    t8a = small.tile([128, 8], F32, name=f"{uniq}_t8a", tag="mg_t8a")
    t8b = small.tile([128, 8], F32, name=f"{uniq}_t8b", tag="mg_t8b")
    m1 = small.tile([128, NG], F32, name=f"{uniq}_m1", tag="mg_m1")
    m2 = small.tile([128, NG], F32, name=f"{uniq}_m2", tag="mg_m2")
    nc.vector.max(t8a[:], vals[:, :width])
    nc.vector.match_replace(m1[:, :width], t8a[:], vals[:, :width], NEG)
    nc.vector.max(t8b[:], m1[:, :width])
    nc.vector.match_replace(m2[:, :width], t8b[:], m1[:, :width], NEG)
    mask = small.tile([128, NG], F32, name=f"{uniq}_mask", tag="mg_mask")
    nc.vector.tensor_scalar(out=mask[:, :width], in0=m2[:, :width],
                            scalar1=-2e38, scalar2=None, op0=AL.is_le)
    csA = small.tile([128, NG], F32, name=f"{uniq}_csA", tag="mg_csA")
    csB = small.tile([128, NG], F32, name=f"{uniq}_csB", tag="mg_csB")
    nc.vector.tensor_copy(csA[:, :width], mask[:, :width])
    src, dst = csA, csB
    sh = 1
    while sh < width:
        nc.vector.tensor_copy(dst[:, 0:sh], src[:, 0:sh])
        nc.vector.tensor_tensor(out=dst[:, sh:width], in0=src[:, sh:width],
                                in1=src[:, 0:width - sh], op=AL.add)
        src, dst = dst, src
        sh *= 2
    rk = small.tile([128, NG], F32, name=f"{uniq}_rk", tag="mg_rk")
    nc.vector.tensor_tensor(out=rk[:, :width], in0=src[:, :width],
                            in1=mask[:, :width], op=AL.mult)
    nc.vector.tensor_scalar(out=rk[:, :width], in0=rk[:, :width], scalar1=-1.0,
                            scalar2=None, op0=AL.add)
    rk16 = small.tile([128, NG], I16, name=f"{uniq}_rk16", tag="mg_rk16")
    nc.vector.tensor_copy(rk16[:, :width], rk[:, :width])
    for plane, out16 in payloads:
        nc.gpsimd.local_scatter(out16[:].bitcast(I16), plane.bitcast(I16),
                                rk16[:, :width], channels=128, num_elems=K,
                                num_idxs=width)


def build():
    nc = bacc.Bacc("TRN2", target_bir_lowering=False, num_devices=NCORES)

    xh_in = nc.dram_tensor("xh", [4, 128, B], U16, kind="ExternalInput")
    xc8_in = nc.dram_tensor("xc8", [4, 128, 2, B], U8, kind="ExternalInput")
    yh_in = nc.dram_tensor("yh", [4, 128, COLS], U16, kind="ExternalInput")
    yc8_in = nc.dram_tensor("yc8", [4, 128, 2, COLS], U8, kind="ExternalInput")
    yn3_in = nc.dram_tensor("yn3", [3, COLS], U16, kind="ExternalInput")
    xn_in = nc.dram_tensor("xn", [128, QTILES], F32, kind="ExternalInput")
    bnd_in = nc.dram_tensor("bnd", [1, NCLASS], F32, kind="ExternalInput")
    out_d = nc.dram_tensor("out", [2 * 128, NCLASS], F32, kind="ExternalOutput")

    ag_in = [nc.dram_tensor(f"ag_in{g}", [GQT * 128, 2 * K], F32)
             for g in range(GROUPS)]
    ag_out = nc.dram_tensor("ag_out", [GROUPS * NCORES * GQT * 128, 2 * K],
                            F32, addr_space="Shared")

    with tile.TileContext(nc) as tc:
        with tc.tile_pool(name="res", bufs=1) as res, \
             tc.tile_pool(name="zps", bufs=2, space="PSUM") as zps, \
             tc.tile_pool(name="candp", bufs=2) as candp, \
             tc.tile_pool(name="small", bufs=2) as small:

            # ---------------- resident tensors ----------------
            xh_r = [res.tile([128, B], F16, name=f"xh{k}") for k in range(4)]
            xc8_r = [res.tile([128, 2, B], F8E4, name=f"xc{k}")
                     for k in range(4)]
            yh_r = [res.tile([128, COLS], F16, name=f"yh{k}") for k in range(4)]
            yc8_r = [res.tile([128, 2, COLS], F8E4, name=f"yc{k}")
                     for k in range(4)]
            yn3 = res.tile([3, COLS], BF16)
            ones3 = res.tile([3, 128], BF16)
            xn_r = res.tile([128, QTILES], F32)
            bnd_f = res.tile([128, NCLASS], F32)
            cio_f = res.tile([128, NCLASS], F32)
            base32 = res.tile([128, NCAND], U16)

            # small constants first
            nc.vector.memset(ones3[:], -1.0)
            nc.sync.dma_start(yn3[:].bitcast(U16), yn3_in[:])
            nc.sync.dma_start(xn_r[:], xn_in[:])
            nc.gpsimd.iota(base32[:], pattern=[[2048, NW], [0, 8]],
                           channel_multiplier=0)
            cio_i = small.tile([128, NCLASS], I32, tag="cioi", bufs=1)
            nc.gpsimd.iota(cio_i[:], pattern=[[1, NCLASS]],
                           channel_multiplier=0)
            nc.vector.tensor_copy(cio_f[:], cio_i[:])
            # broadcast bnd row to 128 partitions via f32 matmul
            bnd_row = small.tile([1, NCLASS], F32, tag="bndrow", bufs=1)
            nc.sync.dma_start(bnd_row[:], bnd_in[:])
            ones1 = small.tile([1, 128], F32, tag="ones1", bufs=1)
            nc.vector.memset(ones1[:], 1.0)
            bps = zps.tile([128, 2048], F32, name="bps", tag="zw")
            nc.tensor.matmul(bps[:, 0:NCLASS], ones1[:], bnd_row[:],
                             start=True, stop=True)
            nc.scalar.copy(bnd_f[:], bps[:, 0:NCLASS])

            # x side (needed by every window)
            for k in range(4):
                nc.sync.dma_start(xh_r[k][:].bitcast(U16), xh_in[k])
                nc.sync.dma_start(xc8_r[k][:].bitcast(U8), xc8_in[k])
            # y side in column blocks so window 0 can start early
            for (a, b) in WIN:
                for k in range(4):
                    nc.sync.dma_start(yh_r[k][:, a:b].bitcast(U16),
                                      yh_in[k, :, a:b])
                    nc.sync.dma_start(yc8_r[k][:, :, a:b].bitcast(U8),
                                      yc8_in[k, :, :, a:b])

            pid_sp = nc.sync.partition_id()

            def global_phase(l):
                """Merge + vote for owned qtile qt = pid + 8*l."""
                # qt = pid + 8l; group g = qt//4; lq = qt%4
                # ag_out row base for core c2: 4096*g + 512*c2 + 128*lq
                gv = small.tile([128, NG], F32, name=f"gv{l}", tag="gv")
                gl = small.tile([128, NG], F32, name=f"gl{l}", tag="gl")
                qt = pid_sp + 8 * l
                # rowoff = 4096*(qt//4) + 128*(qt%4):
                #   pid 0..3 -> qt//4 = 2l (+0), qt%4 = pid
                #   pid 4..7 -> qt//4 = 2l+1, qt%4 = pid-4
                # 4096*(qt//4)+128*(qt%4) = 8192*l + 896*pid for pid<4
                #  and 8192*l + 4096 + 128*(pid-4) = 8192*l + 3584 + 128*pid..
                # Avoid runtime div: rowoff = 8192*l + 896*pid if pid<4 else
                # 8192*l + 3584 + 128*pid. Uniform: note 4096*(pid//4) =
                # 1024*pid - 1024*(pid%4)... simpler: qt//4*4096 + qt%4*128
                # with qt//4 = (pid + 8l) >> 2 and qt%4 = pid & 3.
                g_reg = (pid_sp + 8 * l) // 4
                lq_reg = pid_sp % 4
                for c2 in range(NCORES):
                    base = g_reg * (NCORES * GQT * 128) + c2 * (GQT * 128)
                    nc.sync.dma_start(
                        gv[:, c2 * K:(c2 + 1) * K],
                        ag_out[bass.ds(base + lq_reg * 128, 128), 0:K])
                    nc.sync.dma_start(
                        gl[:, c2 * K:(c2 + 1) * K],
                        ag_out[bass.ds(base + lq_reg * 128, 128), K:2 * K])
                vlo = small.tile([128, NG], U16, name=f"gvlo{l}", tag="vlo")
                vhi = small.tile([128, NG], U16, name=f"gvhi{l}", tag="vhi")
                gvu = gv[:].bitcast(U16).rearrange("p (a two) -> p a two",
                                                   two=2)
                nc.vector.tensor_copy(vlo[:], gvu[:, :, 0:1])
                nc.vector.tensor_copy(vhi[:], gvu[:, :, 1:2])
                glu = small.tile([128, NG], U16, name=f"glu{l}", tag="glu")
                nc.vector.tensor_copy(glu[:], gl[:])
                slo = small.tile([128, K], U16, name=f"gslo{l}", tag="slo16")
                shi = small.tile([128, K], U16, name=f"gshi{l}", tag="shi16")
                sla = small.tile([128, K], U16, name=f"gsla{l}", tag="sgi16")
                _merge_top16(nc, small, f"gm{l}", gv, NG,
                             [(vlo[:], slo), (vhi[:], shi), (glu[:], sla)])
                v16 = small.tile([128, K], F32, name=f"gv16{l}", tag="v16")
                v16u = v16[:].bitcast(U16).rearrange("p (a two) -> p a two",
                                                     two=2)
                nc.vector.tensor_copy(v16u[:, :, 0:1], slo[:])
                nc.vector.tensor_copy(v16u[:, :, 1:2], shi[:])
                lab16 = small.tile([128, K], F32, name=f"glab{l}",
                                   tag="lab16")
                nc.vector.tensor_copy(lab16[:], sla[:])
                xn_col = small.tile([128, 1], F32, name=f"xnc{l}",
                                    tag="xncol")
                nc.sync.dma_start(xn_col[:],
                                  xn_r[:, bass.ds(pid_sp + 8 * l, 1)])
                dsq = small.tile([128, K], F32, name=f"dsq{l}", tag="dsq")
                nc.scalar.activation(dsq[:], v16[:], AF.Sqrt,
                                     scale=-1.0 / BETA, bias=xn_col[:, 0:1])
                ew = small.tile([128, K], F32, name=f"ew{l}", tag="ew")
                zsum = small.tile([128, 1], F32, name=f"zs{l}", tag="zs")
                nc.scalar.activation(ew[:], dsq[:], AF.Exp, scale=-1.0,
                                     accum_out=zsum[:, 0:1])
                rz = small.tile([128, 1], F32, name=f"rz{l}", tag="rz")
                nc.vector.reciprocal(rz[:], zsum[:])
                wt = small.tile([128, K], F32, name=f"wt{l}", tag="wt")
                nc.vector.tensor_scalar(out=wt[:], in0=ew[:],
                                        scalar1=rz[:, 0:1], scalar2=None,
                                        op0=AL.mult)
                vote = small.tile([128, NCLASS], F32, name=f"vote{l}",
                                  tag="vote")
                tmp = small.tile([128, NCLASS], F32, name=f"vtmp{l}",
                                 tag="vtmp")
                nc.vector.memset(vote[:], 0.0)
                for r in range(K):
                    nc.vector.tensor_scalar(out=tmp[:], in0=cio_f[:],
                                            scalar1=lab16[:, r:r + 1],
                                            scalar2=wt[:, r:r + 1],
                                            op0=AL.is_equal, op1=AL.mult)
                    nc.vector.tensor_tensor(out=vote[:], in0=vote[:],
                                            in1=tmp[:], op=AL.add)
                nc.sync.dma_start(out_d[l * 128:(l + 1) * 128, :], vote[:])

            # ---------------- main loop ----------------
            for g in range(GROUPS):
                cands = {}
                for lq in range(GQT):
                    cands[lq] = (
                        candp.tile([128, NCAND], F32, name=f"cv{g}_{lq}",
                                   tag=f"cv{lq}"),
                        candp.tile([128, NCAND], U16, name=f"ci{g}_{lq}",
                                   tag=f"ci{lq}"),
                    )
                for w, (a, b) in enumerate(WIN):
                    wlen = b - a
                    for lq in range(GQT):
                        qt = g * GQT + lq
                        qs = qt * 128
                        ps = zps.tile([128, 2048], F32,
                                      name=f"ps{g}_{w}_{lq}", tag="zw")
                        for co0 in range(0, wlen, 512):
                            co = a + co0
                            cw = min(512, b - co)
                            pslice = ps[:, co0:co0 + cw]
                            nc.tensor.matmul(pslice, ones3[:],
                                             yn3[:, co:co + cw],
                                             start=True, stop=False)
                            for k in range(4):
                                nc.tensor.matmul(pslice,
                                                 xh_r[k][:, qs:qs + 128],
                                                 yh_r[k][:, co:co + cw],
                                                 start=False, stop=False)
                            for k in range(4):
                                nc.tensor.matmul(
                                    pslice,
                                    xc8_r[k][:, :, qs:qs + 128],
                                    yc8_r[k][:, :, co:co + cw],
                                    start=False, stop=(k == 3),
                                    perf_mode=DR)
                        cv, ci = cands[lq]
                        nc.vector.max(cv[:, w * 8:w * 8 + 8], ps[:, :wlen])
                        nc.vector.max_index(ci[:, w * 8:w * 8 + 8],
                                            cv[:, w * 8:w * 8 + 8],
                                            ps[:, :wlen])

                # owned qtile qt=pid sits in group 0 or 1; AG_1 has had all
                # of group 2's scans to land, so this inserts no engine wait
                if g == 2:
                    global_phase(0)

                for lq in range(GQT):
                    qt = g * GQT + lq
                    cv, ci = cands[lq]
                    gi = small.tile([128, NCAND], U16, name=f"gi{qt}",
                                    tag="gi")
                    nc.vector.tensor_tensor(out=gi[:], in0=ci[:],
                                            in1=base32[:], op=AL.add)
                    vlo = small.tile([128, NCAND], U16, name=f"vlo{qt}",
                                     tag="vlo")
                    vhi = small.tile([128, NCAND], U16, name=f"vhi{qt}",
                                     tag="vhi")
                    cvu = cv[:].bitcast(U16).rearrange(
                        "p (a two) -> p a two", two=2)
                    nc.vector.tensor_copy(vlo[:, :NCAND], cvu[:, :, 0:1])
                    nc.vector.tensor_copy(vhi[:, :NCAND], cvu[:, :, 1:2])
                    slo = small.tile([128, K], U16, name=f"slo16_{qt}",
                                     tag="slo16")
                    shi = small.tile([128, K], U16, name=f"shi16_{qt}",
                                     tag="shi16")
                    sgi = small.tile([128, K], U16, name=f"sgi16_{qt}",
                                     tag="sgi16")
                    _merge_top16(nc, small, f"lm{qt}", cv, NCAND,
                                 [(vlo[:, :NCAND], slo),
                                  (vhi[:, :NCAND], shi), (gi[:], sgi)])
                    v16 = small.tile([128, K], F32, name=f"v16_{qt}",
                                     tag="v16")
                    v16u = v16[:].bitcast(U16).rearrange(
                        "p (a two) -> p a two", two=2)
                    nc.vector.tensor_copy(v16u[:, :, 0:1], slo[:])
                    nc.vector.tensor_copy(v16u[:, :, 1:2], shi[:])
                    # labels: idx -> sum of sign(idx + 0.5 - bnd_k) over k,
                    # then label = 0.5*sum + 49.5 - 0.5 = (sum+98)/2? see
                    # host: bnd_k <= idx for k <= label -> sum = 2*label-98.
                    gfh = small.tile([128, K], F32, name=f"gfh{qt}",
                                     tag="gfh")
                    nc.vector.tensor_scalar(out=gfh[:], in0=sgi[:],
                                            scalar1=0.5, scalar2=None,
                                            op0=AL.add)
                    junk = small.tile([128, NCLASS], F32, name=f"sj{qt}",
                                      tag="sjunk")
                    labsum = small.tile([128, K], F32, name=f"ls{qt}",
                                        tag="labsum")
                    for r in range(K):
                        nc.scalar.activation(junk[:], bnd_f[:], AF.Sign,
                                             bias=gfh[:, r:r + 1],
                                             scale=-1.0,
                                             accum_out=labsum[:, r:r + 1])
                    lab16 = small.tile([128, K], F32, name=f"lab{qt}",
                                       tag="lab16l")
                    nc.vector.tensor_scalar(out=lab16[:], in0=labsum[:],
                                            scalar1=0.5, scalar2=49.0,
                                            op0=AL.mult, op1=AL.add)
                    nc.sync.dma_start(
                        ag_in[g][lq * 128:(lq + 1) * 128, 0:K], v16[:])
                    nc.sync.dma_start(
                        ag_in[g][lq * 128:(lq + 1) * 128, K:2 * K], lab16[:])

                nc.gpsimd.collective_compute(
                    "AllGather", AL.bypass,
                    replica_groups=[list(range(NCORES))],
                    ins=[ag_in[g][:].opt()],
                    outs=[ag_out[g * NCORES * GQT * 128:
                                 (g + 1) * NCORES * GQT * 128, :].opt()])

                if g == 3:
                    global_phase(1)

    nc.finalize()
    return nc


_NC_CACHE = None


def _e4m3(a):
    return np.clip(a, -240.0, 240.0).astype(ml_dtypes.float8_e4m3fn)


def _prep_host(x, tf, tl):
    """Build per-core input maps (host-side marshalling)."""
    x = np.ascontiguousarray(np.asarray(x, dtype=np.float32))
    tf = np.ascontiguousarray(np.asarray(tf, dtype=np.float32))
    tl = np.asarray(tl, dtype=np.int64)

    X = 1024.0 * x
    Xh16 = X.astype(np.float16)
    Xh = Xh16.astype(np.float32)
    Xl = X - Xh
    A_lhs = _e4m3(Xh / 512.0)            # [B, 512] fp8
    B_lhs = _e4m3(Xl)

    # transposed k-subtile blocks
    xh_t = np.ascontiguousarray(
        Xh16.T.reshape(4, 128, B).view(np.uint16))
    xc8 = np.ascontiguousarray(np.stack(
        [A_lhs.T.reshape(4, 128, B).view(np.uint8),
         B_lhs.T.reshape(4, 128, B).view(np.uint8)],
        axis=2))                          # [4, 128, 2, B] u8

    xn = (x * x).sum(1).astype(np.float32).reshape(QTILES, 128).T
    xn = np.ascontiguousarray(xn)         # [128, 16]

    in_maps = []
    for c in range(NCORES):
        sl = slice(c * NSHARD, (c + 1) * NSHARD)
        labs = tl[sl]
        feats = tf[sl]
        perm = np.argsort(labs, kind="stable")
        feats = np.ascontiguousarray(feats[perm])
        labs_s = labs[perm]
        bnd = np.searchsorted(labs_s, np.arange(NCLASS), side="left")

        y = np.zeros((COLS, D), np.float32)
        y[:NSHARD] = feats
        yn = np.full(COLS, BIG, np.float32)
        yn[:NSHARD] = (feats * feats).sum(1)

        Yh16 = y.astype(np.float16)
        Yh = Yh16.astype(np.float32)
        Yl = y - Yh
        A_rhs = _e4m3(512.0 * Yl)
        B_rhs = _e4m3(Yh)

        yh_t = np.ascontiguousarray(
            Yh16.T.reshape(4, 128, COLS).view(np.uint16))
        yc8 = np.ascontiguousarray(np.stack(
            [A_rhs.T.reshape(4, 128, COLS).view(np.uint8),
             B_rhs.T.reshape(4, 128, COLS).view(np.uint8)],
            axis=2))                      # [4, 128, 2, COLS] u8

        v = (BETA * yn).astype(np.float32)
        y1 = v.astype(ml_dtypes.bfloat16)
        y2 = (v - y1.astype(np.float32)).astype(ml_dtypes.bfloat16)
        y3 = (v - y1.astype(np.float32) - y2.astype(np.float32)).astype(
            ml_dtypes.bfloat16)
        yn3 = np.ascontiguousarray(
            np.stack([y1, y2, y3], axis=0).view(np.uint16))

        in_maps.append({
            "xh": xh_t,
            "xc8": xc8,
            "yh": yh_t,
            "yc8": yc8,
            "yn3": yn3,
            "xn": xn,
            "bnd": bnd.astype(np.float32)[None, :],
        })
    return in_maps


def kernel(x, train_features, train_labels, **run_kwargs):
    global _NC_CACHE
    in_maps = _prep_host(x, train_features, train_labels)
    if _NC_CACHE is None:
        _NC_CACHE = build()
    res = bass_utils.run_bass_kernel_spmd(
        _NC_CACHE, in_maps, core_ids=list(range(NCORES)), **run_kwargs)
    global LAST_RESULTS
    LAST_RESULTS = res
    out = np.zeros((B, NCLASS), np.float32)
    for c in range(NCORES):
        o = res.results[c]["out"]
        out[c * 128:(c + 1) * 128] = o[0:128]
        out[(8 + c) * 128:(9 + c) * 128] = o[128:256]
    return out.astype(np.float32)


LAST_RESULTS = None
